# revision 2
# baseline (speedup 1.0000x reference)
"""Trainium2 kernel for nn_Direction: out = input @ Q.T, Q from QR(weight + 1e-8).

Strategy:
  - Host: QR of the small 512x512 weight (LAPACK/jax fp32), pre-transpose each
    batch shard so the contraction dim (motion=512) lands on SBUF partitions,
    cast to fp16.
  - Device (8 cores, data-parallel over batch): tiled matmul, single fp16 pass
    (rel err ~3e-4 vs fp32 reference, well under the 2e-2 gate), accumulated in
    PSUM over 4 k-tiles per 128-batch tile. A.T tiles are the stationary
    operand, Q.T tiles are SBUF-resident constants streamed as the moving
    operand (N=512). PSUM is evicted to fp16 (cast on vector/scalar engines)
    to halve output DMA traffic; host upcasts to fp32.
  - Gather: concatenate the 8 output shards.

Roofline: per core 512 matmuls x 512 cols @ 2.4 GHz = ~110 us PE-bound;
fp16 in+out DMA = 32 MB @ ~358 GB/s = ~89 us, overlapped under compute.
"""

import numpy as np

import concourse.bacc as bacc
import concourse.mybir as mybir
import concourse.tile as tile
from concourse.bass_utils import run_bass_kernel_spmd

B_FULL = 131072
D = 512
N_CORES = 8
B_LOC = B_FULL // N_CORES  # 16384
P = 128
BT = 512  # batch rows per loop iteration
KT = D // P  # 4 k-tiles
SB = BT // P  # 4 psum sub-tiles per iteration

# precision mode: "fp32" | "fp16" | "bf16" | "fp16x2"
MODE = "fp16"

_DT = {
    "fp32": mybir.dt.float32,
    "fp16": mybir.dt.float16,
    "bf16": mybir.dt.bfloat16,
    "fp16x2": mybir.dt.float16,
}

# (a_input, q_input) matmul passes, accumulated in PSUM.
_PASSES = {
    "fp32": [("a0", "q0")],
    "fp16": [("a0", "q0")],
    "bf16": [("a0", "q0")],
    "fp16x2": [("a0", "q0"), ("a1", "q0"), ("a0", "q1")],
}

_CACHE = {}


def _build(mode, b_loc, reps=1, dynamic=False, bt=BT, ain_bufs=4, aout_bufs=4,
           ps_bufs=8, evict="alt", out16=True):
    dt_in = _DT[mode]
    dt_out = mybir.dt.float16 if out16 else mybir.dt.float32
    passes = _PASSES[mode]
    a_names = sorted({a for a, _ in passes})
    q_names = sorted({q for _, q in passes})
    n_iter = b_loc // bt
    sb_n = bt // P

    nc = bacc.Bacc("TRN2", target_bir_lowering=False, debug=False)
    a_dram = {
        n: nc.dram_tensor(n, [D, b_loc], dt_in, kind="ExternalInput").ap()
        for n in a_names
    }
    q_dram = {
        n: nc.dram_tensor(n, [D, D], dt_in, kind="ExternalInput").ap()
        for n in q_names
    }
    out_dram = nc.dram_tensor(
        "out", [b_loc, D], dt_out, kind="ExternalOutput"
    ).ap()

    with tile.TileContext(nc) as tc:
        with (
            tc.tile_pool(name="consts", bufs=1) as consts,
            tc.tile_pool(name="ain", bufs=ain_bufs) as ain,
            tc.tile_pool(name="aout", bufs=aout_bufs) as aout,
            tc.tile_pool(name="ps", bufs=ps_bufs, space="PSUM") as ps_pool,
        ):
            q_tiles = {}
            for qn in q_names:
                qt = consts.tile([P, KT, D], dt_in, name=f"qt_{qn}")
                nc.sync.dma_start(
                    out=qt[:, :, :],
                    in_=q_dram[qn].rearrange("(k p) n -> p k n", p=P),
                )
                q_tiles[qn] = qt

            def body():
                for it in range(n_iter):
                    a_tiles = {}
                    for an in a_names:
                        at = ain.tile(
                            [P, KT, bt], dt_in, name=f"at_{an}", tag=f"at_{an}"
                        )
                        src = a_dram[an].rearrange("(k p) b -> p k b", p=P)[
                            :, :, it * bt : (it + 1) * bt
                        ]
                        nc.sync.dma_start(out=at[:, :, :], in_=src)
                        a_tiles[an] = at
                    for sb in range(sb_n):
                        ps = ps_pool.tile(
                            [P, D], mybir.dt.float32, name="ps", tag="ps"
                        )
                        n_mm = len(passes) * KT
                        mm = 0
                        for an, qn in passes:
                            at = a_tiles[an]
                            qt = q_tiles[qn]
                            for k in range(KT):
                                nc.tensor.matmul(
                                    ps[:, :],
                                    at[:, k, sb * P : (sb + 1) * P],
                                    qt[:, k, :],
                                    start=(mm == 0),
                                    stop=(mm == n_mm - 1),
                                )
                                mm += 1
                        ot = aout.tile([P, D], dt_out, name="ot", tag="ot")
                        if evict == "any":
                            nc.any.tensor_copy(ot[:, :], ps[:, :])
                        elif evict == "vector":
                            nc.vector.tensor_copy(ot[:, :], ps[:, :])
                        elif evict == "alt":
                            if sb % 2 == 0:
                                nc.vector.tensor_copy(ot[:, :], ps[:, :])
                            else:
                                nc.scalar.activation(
                                    ot[:, :],
                                    ps[:, :],
                                    mybir.ActivationFunctionType.Copy,
                                )
                        b0 = it * bt + sb * P
                        nc.sync.dma_start(out=out_dram[b0 : b0 + P, :], in_=ot[:, :])

            if dynamic == "unroll" and reps > 1:
                tc.For_i_unrolled(0, reps, 1, lambda iv: body(), max_unroll=4)
            elif dynamic and reps > 1:
                with tc.For_i(0, reps, 1):
                    body()
            else:
                for _ in range(reps):
                    body()

    nc.compile()
    return nc


def _get_nc(mode, b_loc, **kw):
    return _get_nc_reps(mode, b_loc, 1, **kw)


def _get_nc_reps(mode, b_loc, reps, dynamic=False, **kw):
    key = (mode, b_loc, reps, dynamic, tuple(sorted(kw.items())))
    if key not in _CACHE:
        _CACHE[key] = _build(mode, b_loc, reps, dynamic, **kw)
    return _CACHE[key]


def _split16(x):
    hi = x.astype(np.float16)
    lo = (x - hi.astype(np.float32)).astype(np.float16)
    return hi, lo


def _prep_inputs(mode, input_np, qt_np, n_cores, b_loc):
    """Build per-core input maps. input_np: (n_cores*b_loc, D) fp32 row-major.
    qt_np: (D, D) fp32, qt_np[m, n] = Q[n, m]."""
    maps = []
    if mode == "fp16x2":
        qh, ql = _split16(qt_np)
        for i in range(n_cores):
            at = np.ascontiguousarray(input_np[i * b_loc : (i + 1) * b_loc].T)
            ah, al = _split16(at)
            maps.append({"a0": ah, "a1": al, "q0": qh, "q1": ql})
    else:
        if mode == "bf16":
            import ml_dtypes

            cast_dt = ml_dtypes.bfloat16
        else:
            cast_dt = {"fp32": np.float32, "fp16": np.float16}[mode]
        q0 = qt_np.astype(cast_dt)
        # cast first (vectorized over the full row-major array), then
        # transpose per-core shards
        inp_c = input_np.astype(cast_dt)
        for i in range(n_cores):
            at = np.ascontiguousarray(inp_c[i * b_loc : (i + 1) * b_loc].T)
            maps.append({"a0": at, "q0": q0})
    return maps


def _compute_qt(weight_np):
    """Q from QR(weight + 1e-8), transposed. Prefer jax-on-CPU so Q matches the
    fp32 jax reference bit-for-bit when possible; fall back to LAPACK (both are
    Householder QR and agree to ~1e-6, so either is well within tolerance)."""
    w = weight_np.astype(np.float32)
    try:
        import jax
        import jax.numpy as jnp

        cpu = jax.devices("cpu")[0]
        with jax.default_device(cpu):
            q, _ = jnp.linalg.qr(jax.device_put(w, cpu) + 1e-8)
        q = np.asarray(q)
    except Exception:
        q, _ = np.linalg.qr(w + np.float32(1e-8))
    return np.ascontiguousarray(q.T.astype(np.float32))


def run(input_np, weight_np, mode=None, n_cores=N_CORES, b_loc=None,
        build_kw=None, **run_kwargs):
    mode = mode or MODE
    b_loc = b_loc or (input_np.shape[0] // n_cores)
    assert input_np.shape[0] == n_cores * b_loc, (
        f"batch {input_np.shape[0]} not divisible into {n_cores} cores"
    )
    assert b_loc % BT == 0 and input_np.shape[1] == D

    qt = _compute_qt(weight_np)

    nc = _get_nc(mode, b_loc, **(build_kw or {}))
    in_maps = _prep_inputs(mode, np.asarray(input_np), qt, n_cores, b_loc)
    res = run_bass_kernel_spmd(nc, in_maps, list(range(n_cores)), **run_kwargs)
    out = np.concatenate(
        [np.asarray(res.results[i]["out"], dtype=np.float32)
         for i in range(n_cores)],
        axis=0,
    )
    return out, res


def kernel(input, weight):
    out, _ = run(
        np.asarray(input, dtype=np.float32), np.asarray(weight, dtype=np.float32)
    )
    return np.ascontiguousarray(out, dtype=np.float32)
